# revision 8
# baseline (speedup 1.0000x reference)
"""BERT self-attention (B=8, S=1024, D=768, H=12) on 8 TRN2 NeuronCores.

Strategy
--------
Data-parallel over batch: core b handles batch element b (no collectives).

Per core, Q=K=V makes the score matrix symmetric, which this kernel
exploits end to end:

  1. mixT[e, s] = W^T x^T + b as bf16 matmuls with fp32 psum accumulation,
     evacuated per head into [80, 1024] tiles whose row 64 is a constant
     ones row (rows 65..79 are don't-care padding for the DMA transpose).
  2. xl[s, chunk, 0:65] (per-head values + ones column for the softmax
     denominator) is produced by a single DMA xbar transpose of the [80,
     1024] mix tile -- no PE transposes, no DVE copies.
  3. Scores are computed only for the upper triangle: for t-chunk i, a
     K=64 matmul covers s >= 128*i (36 of 64 blocks per head), and exp is
     applied only there (43% less ACT work, the kernel's old bottleneck).
  4. The strictly-lower exp'd blocks are mirrors of the upper ones:
     they are materialized by DMA xbar transposes (u row j -> staging
     tile um) which run on the otherwise-idle DMA engines, off the PE.
  5. ctx'^T accumulates per 512-col half over the 8 t-chunks, with the
     rhs taken from the exp'd tiles (upper blocks) or um (mirrors); the
     ones column of xl gives the denominator as psum row 64.
  6. Epilogue per 128-col chunk: PE transpose, DVE reciprocal of the
     denominator column, gpsimd per-partition scale into staging tiles,
     flushed as large contiguous DMAs (gpsimd queue).

The attention mask in this problem is identically zero (spec fill) and
exp(0 + score) keeps the mirror blocks exact; a nonzero mask would break
the mirror trick, so kernel() falls back to a host reference in that
(never exercised) case.

Emission order interleaves each head's (gated) score matmuls with the
previous head's ctx/epilogue so the PE never idles behind the ACT
engine; the final head recomputes its full score rows (no mirrors) so
its ctx matmuls interleave with its own exps and only the epilogue
trails the last ACTIVATE.
"""

import numpy as np

import concourse.bacc as bacc
import concourse.tile as tile
from concourse import mybir
from concourse.bass_utils import run_bass_kernel_spmd
from concourse.masks import make_identity

B, S, D = 8, 1024, 768
H, DH = 12, 64
NP = 6            # e-tile pairs (2 heads each)
NT = 8            # t-chunks / s-chunks of 128
F32 = mybir.dt.float32
BF16 = mybir.dt.bfloat16
EXP = mybir.ActivationFunctionType.Exp

# um slot offsets: mirrors of source row j occupy slots [MOFF[j], MOFF[j]+7-j)
MOFF = [0]
for _j in range(6):
    MOFF.append(MOFF[-1] + (7 - _j))

_CACHED_NC = None


def build_nc():
    nc = bacc.Bacc("TRN2", target_bir_lowering=False)

    xT = nc.dram_tensor("xT", [D, S], BF16, kind="ExternalInput")
    wT = nc.dram_tensor("wT", [D, D], BF16, kind="ExternalInput")
    bias_d = nc.dram_tensor("bias_d", [128, NP], F32, kind="ExternalInput")
    out_d = nc.dram_tensor("out", [S, D], F32, kind="ExternalOutput")

    with tile.TileContext(nc) as tc:
        with (
            tc.tile_pool(name="consts", bufs=1) as consts,
            tc.tile_pool(name="big", bufs=1) as big,
            tc.tile_pool(name="upool", bufs=18) as upool,
            tc.tile_pool(name="umpool", bufs=2) as umpool,
            tc.tile_pool(name="ctpool", bufs=4) as ctpool,
            tc.tile_pool(name="rpool", bufs=24) as rpool,
            tc.tile_pool(name="ps_big", bufs=2, space="PSUM") as ps_big,
            tc.tile_pool(name="ps_small", bufs=2, space="PSUM") as ps_small,
        ):
            identbf = consts.tile([128, 128], BF16)
            make_identity(nc, identbf)

            wts = big.tile([128, NP, D], BF16)
            xts = big.tile([128, NP, S], BF16)
            for k in range(NP):
                eng = nc.sync if k % 2 == 0 else nc.scalar
                eng.dma_start(out=xts[:, k, :],
                              in_=xT[k * 128:(k + 1) * 128, :])
                nc.gpsimd.dma_start(out=wts[:, k, :],
                                    in_=wT[k * 128:(k + 1) * 128, :])
            bias_t = consts.tile([128, NP], F32)
            nc.gpsimd.dma_start(out=bias_t, in_=bias_d[:, :])

            # Preload the ACT exp table while the inputs stream in.
            warm = consts.tile([128, 16], F32)
            nc.scalar.activation(out=warm, in_=identbf[:, 0:16],
                                 func=EXP, scale=0.125)

            mixh = [big.tile([80, S], BF16, name=f"mix{h}") for h in range(H)]
            xl80 = [big.tile([128, NT, 80], BF16, name=f"xl{h}")
                    for h in range(H)]
            stages = [big.tile([128, H, DH], F32, name=f"stage{sj}")
                      for sj in range(NT)]
            for h in range(H):
                nc.gpsimd.memset(mixh[h][64:80, :], 1.0)

            def proj(p, warmup=False):
                """Projection pair p -> mixh[2p], mixh[2p+1] + xl transposes."""
                for nn in range(2):
                    pm = ps_small.tile([128, 512], F32, name="pm", tag="sc")
                    for k in range(NP):
                        nc.tensor.matmul(
                            pm,
                            lhsT=wts[:, k, p * 128:(p + 1) * 128],
                            rhs=xts[:, k, nn * 512:(nn + 1) * 512],
                            start=(k == 0),
                            stop=(k == NP - 1),
                        )
                        if warmup:
                            # Keep the HAM clock gate open through the
                            # DMA-paced input-streaming window.
                            for _ in range(3):
                                ptw = ps_small.tile([128, 128], BF16,
                                                    name="ptw", tag="pc")
                                nc.tensor.transpose(ptw, identbf, identbf)
                    for q in range(2):
                        nc.vector.tensor_scalar_add(
                            mixh[2 * p + q][0:64, nn * 512:(nn + 1) * 512],
                            pm[q * 64:(q + 1) * 64, :],
                            bias_t[q * 64:(q + 1) * 64, p:p + 1],
                        )
                for q in range(2):
                    nc.sync.dma_start_transpose(out=xl80[2 * p + q],
                                                in_=mixh[2 * p + q])

            def sc_chunk(h, i, full):
                """Scores matmul + exp for (head h, t-chunk i)."""
                w = S if full else (NT - i) * 128
                off = 0 if full else i * 128
                mix = mixh[h]
                psc = ps_big.tile([128, S], F32, name="psc")
                pos = 0
                while pos < w:
                    seg = min(512, w - pos)
                    nc.tensor.matmul(
                        psc[:, pos:pos + seg],
                        lhsT=mix[0:64, i * 128:(i + 1) * 128],
                        rhs=mix[0:64, off + pos:off + pos + seg],
                        start=True, stop=True,
                    )
                    pos += seg
                u = upool.tile([128, S], BF16, name="u")
                nc.scalar.activation(out=u[:, off:off + w], in_=psc[:, 0:w],
                                     func=EXP, scale=0.125)
                return u

            def mirror(i, u, um):
                """Mirror exp'd row i into um via the DMA xbar transpose."""
                if i < NT - 1:
                    nc.sync.dma_start_transpose(
                        out=um[:, MOFF[i]:MOFF[i] + (NT - 1 - i), :],
                        in_=u[:, (i + 1) * 128:S],
                    )

            def ctx_half(h, nn, us, um):
                """ctx'^T accumulation for 512-col half nn of head h."""
                xl = xl80[h]
                pc = ps_small.tile([DH + 1, 512], F32, name="pc", tag="pc")
                lo, hi = 4 * nn, 4 * nn + 4
                for i in range(NT):
                    lhsT = xl[:, i, 0:DH + 1]
                    st, sp = (i == 0), (i == NT - 1)
                    for jout in range(lo, min(i, hi)):
                        slot = MOFF[jout] + (i - jout - 1)
                        nc.tensor.matmul(
                            pc[:, (jout - lo) * 128:(jout - lo + 1) * 128],
                            lhsT=lhsT, rhs=um[:, slot, :],
                            start=st, stop=sp,
                        )
                    a = max(i, lo)
                    if a < hi:
                        nc.tensor.matmul(
                            pc[:, (a - lo) * 128:(hi - lo) * 128],
                            lhsT=lhsT, rhs=us[i][:, a * 128:hi * 128],
                            start=st, stop=sp,
                        )
                return pc

            def epil_half(h, nn, pc):
                """Transpose + normalize one ctx half into the stage tiles."""
                ct = ctpool.tile([DH + 1, 512], BF16, name="ct")
                nc.vector.tensor_copy(out=ct, in_=pc)
                for sjh in range(4):
                    sj = nn * 4 + sjh
                    po = ps_small.tile([128, DH + 1], BF16, name="po",
                                       tag="sc")
                    nc.tensor.transpose(
                        po, ct[:, sjh * 128:(sjh + 1) * 128],
                        identbf[0:DH + 1, 0:DH + 1],
                    )
                    rcol = rpool.tile([128, 1], F32, name="rcol")
                    nc.vector.reciprocal(out=rcol, in_=po[:, DH:DH + 1])
                    nc.vector.tensor_scalar_mul(
                        stages[sj][:, h, :], po[:, 0:DH], rcol
                    )

            def flush(h0, h1, engs):
                for sj in range(NT):
                    engs[sj % len(engs)].dma_start(
                        out=out_d[sj * 128:(sj + 1) * 128, h0 * 64:h1 * 64],
                        in_=stages[sj][:, h0:h1, :],
                    )

            proj(0, warmup=True)
            pending = None
            done_heads = 0
            for h in range(H - 1):
                um = umpool.tile([128, 28, 128], BF16, name="um")
                us = []

                def sc2(i0, h=h, us=us, um=um):
                    for i in (i0, i0 + 1):
                        u = sc_chunk(h, i, False)
                        us.append(u)
                        mirror(i, u, um)

                sc2(0)
                if pending is not None:
                    ph, pus, pum = pending
                    epil_half(ph, 0, ctx_half(ph, 0, pus, pum))
                elif h == 0:
                    proj(1)
                sc2(2)
                if pending is not None:
                    epil_half(ph, 1, ctx_half(ph, 1, pus, pum))
                    done_heads += 1
                    if done_heads == 6:
                        flush(0, 6, [nc.gpsimd])
                elif h == 0:
                    proj(2)
                sc2(4)
                if h == 1:
                    proj(3)
                elif h == 3:
                    proj(4)
                elif h == 5:
                    proj(5)
                sc2(6)
                pending = (h, us, um)

            # Final head: full-width scores (no mirrors) so the ctx matmuls
            # interleave with the exps and only the epilogue trails the
            # last ACTIVATE.
            hL = H - 1
            us = [sc_chunk(hL, i, True) for i in range(NT - 1)]
            ph, pus, pum = pending
            epil_half(ph, 0, ctx_half(ph, 0, pus, pum))
            epil_half(ph, 1, ctx_half(ph, 1, pus, pum))
            flush(6, 10, [nc.gpsimd])
            pcs = [ps_small.tile([DH + 1, 512], F32, name="pc", tag="pc")
                   for _ in range(2)]
            for i in range(NT - 1):
                for nn in range(2):
                    nc.tensor.matmul(
                        pcs[nn],
                        lhsT=xl80[hL][:, i, 0:DH + 1],
                        rhs=us[i][:, nn * 512:(nn + 1) * 512],
                        start=(i == 0), stop=False,
                    )
            u7 = sc_chunk(hL, NT - 1, True)
            for nn in range(2):
                nc.tensor.matmul(
                    pcs[nn],
                    lhsT=xl80[hL][:, NT - 1, 0:DH + 1],
                    rhs=u7[:, nn * 512:(nn + 1) * 512],
                    start=False, stop=True,
                )
            flush(10, 11, [nc.gpsimd])
            for nn in range(2):
                epil_half(hL, nn, pcs[nn])
            flush(11, 12, [nc.sync, nc.gpsimd, nc.scalar])

    nc.compile()
    return nc


def _reference_fallback(x, attention_mask, W, b):
    mixed = np.einsum('bsd,ed->bse', x, W) + b
    xl = mixed.reshape(B, S, H, DH).transpose(0, 2, 1, 3)
    sc = np.einsum('bhsd,bhtd->bhst', xl, xl) / np.sqrt(DH)
    sc = sc + attention_mask
    sc -= sc.max(axis=-1, keepdims=True)
    p = np.exp(sc)
    p /= p.sum(axis=-1, keepdims=True)
    ctx = np.einsum('bhst,bhtd->bhsd', p, xl)
    return ctx.transpose(0, 2, 1, 3).reshape(B, S, D).astype(np.float32)


def kernel(x, attention_mask, W, b, _profile=None):
    global _CACHED_NC

    x = np.asarray(x, dtype=np.float32)
    attention_mask = np.asarray(attention_mask, dtype=np.float32)
    W = np.asarray(W, dtype=np.float32)
    b = np.asarray(b, dtype=np.float32)

    if np.any(attention_mask != 0.0):
        # The symmetric-mirror trick assumes a zero additive mask (the
        # spec fill); fall back to an exact host path otherwise.
        return _reference_fallback(x, attention_mask, W, b)

    if _CACHED_NC is None:
        _CACHED_NC = build_nc()
    nc = _CACHED_NC

    import ml_dtypes

    wTb = np.ascontiguousarray(W.T).astype(ml_dtypes.bfloat16)
    bias_cols = np.ascontiguousarray(b.reshape(NP, 128).T)

    in_maps = []
    for i in range(B):
        in_maps.append({
            "xT": np.ascontiguousarray(x[i].T).astype(ml_dtypes.bfloat16),
            "wT": wTb,
            "bias_d": bias_cols,
        })

    kwargs = dict(_profile) if _profile else {}
    res = run_bass_kernel_spmd(nc, in_maps, core_ids=list(range(B)), **kwargs)
    out = np.stack([res.results[i]["out"] for i in range(B)], axis=0)
    if _profile:
        kernel.last_results = res
    return out


if __name__ == "__main__":
    rng = np.random.default_rng(0)
    x = rng.standard_normal((B, S, D), dtype=np.float32)
    m = np.zeros((B, 1, 1, S), dtype=np.float32)
    W = (rng.standard_normal((D, D), dtype=np.float32) / np.sqrt(D)).astype(np.float32)
    b = np.zeros((D,), dtype=np.float32)
    out = kernel(x, m, W, b)
    print("out", out.shape, out.dtype)


# revision 22
# speedup vs baseline: 1.6282x; 1.6282x over previous
"""BERT self-attention (B=8, S=1024, D=768, H=12) on 8 TRN2 NeuronCores.

Strategy
--------
Data-parallel over batch: core b handles batch element b (no collectives).

Per core, everything is computed in a "transposed" layout that keeps the
tensor engine's contraction dimension in the partition axis:

  1. mixedT[e, s] = sum_d W^T[d, e] * x^T[d, s] + bias[e] as bf16 matmuls
     with fp32 psum accumulation; the psum evacuation adds the
     per-partition bias and keeps bf16 for the attention stage.
  2. Q=K=V => the score matrix is symmetric: scores[t, s] = scores[s, t].
     The exp'd score tile in [t, s] layout therefore equals the transposed
     (unnormalized) probability matrix needed as the moving operand of the
     context matmul -- no probability transposes at all.
     scores tile = Z_h^T @ MIX where Z_h zero-masks the other head of the
     e-tile pair (kills cross-head terms while keeping K=128
     partition-aligned bf16 matmuls at 1 cycle/column).
     U = exp(0.125 * scores + mask[t]) is fused into the ACT psum
     evacuation (the mask enters as the per-partition bias).
  3. ctx'^T[dh, s] plus the softmax denominator as row 64 (via a constant
     ones column in the stationary operand [xl | 1] [128, 65]) accumulate
     in fp32 psum over the eight t-chunks, moving U.
  4. PE-transpose of ctx'^T 128-column chunks gives ctx[s, dh] with the
     denominator as column 64; reciprocal + per-partition scalar multiply
     normalize during the psum evacuation, writing into per-s-chunk
     staging tiles that are flushed as a few large contiguous DMAs.

Scheduling: the kernel is software-pipelined at two levels. The scalar
engine (softmax exp: 96 x [128, 1024] ACTIVATEs, ~107us) is the
steady-state bottleneck, so emission order makes everything else fill its
bubbles: scores+exp of head h are emitted before the ctx/output phase of
head h-1, and the projection/prep of e-tile pair j+1 is emitted between
them (so its evac->z chain outranks the ctx epilogues and the next pair's
scores are ready the moment this pair's exps drain). The final head's ctx
matmuls are interleaved with its own exps so only the short epilogue
trails the last ACTIVATE.

Measured on TRN2 (8 cores): ~138.2 us HW exec, rel err ~8e-3 vs the fp32
reference (bf16 operand rounding; all accumulation in fp32).
"""

import numpy as np

import concourse.bacc as bacc
import concourse.tile as tile
from concourse import mybir
from concourse.bass_utils import run_bass_kernel_spmd
from concourse.masks import make_identity

B, S, D = 8, 1024, 768
H, DH = 12, 64
NP = 6            # e-tile pairs (2 heads each)
NT = 8            # t-chunks / s-chunks of 128
F32 = mybir.dt.float32
F32R = mybir.dt.float32r
BF16 = mybir.dt.bfloat16
EXP = mybir.ActivationFunctionType.Exp

_CACHED_NC = None


def build_nc():
    nc = bacc.Bacc("TRN2", target_bir_lowering=False)

    xT = nc.dram_tensor("xT", [D, S], BF16, kind="ExternalInput")
    wT = nc.dram_tensor("wT", [D, D], BF16, kind="ExternalInput")
    bias_d = nc.dram_tensor("bias_d", [128, NP], F32, kind="ExternalInput")
    mask_d = nc.dram_tensor("mask_d", [128, NT], F32, kind="ExternalInput")
    out_d = nc.dram_tensor("out", [S, D], F32, kind="ExternalOutput")

    with tile.TileContext(nc) as tc:
        with (
            tc.tile_pool(name="consts", bufs=1) as consts,
            tc.tile_pool(name="big", bufs=1) as big,
            tc.tile_pool(name="upool", bufs=22) as upool,
            tc.tile_pool(name="ctpool", bufs=4) as ctpool,
            tc.tile_pool(name="rpool", bufs=24) as rpool,
            tc.tile_pool(name="ps_s", bufs=2, space="PSUM") as ps_s,
            tc.tile_pool(name="ps_c", bufs=1, space="PSUM") as ps_c,
            tc.tile_pool(name="ps_t", bufs=2, space="PSUM") as ps_t,
        ):
            ident32 = consts.tile([128, 128], F32)
            make_identity(nc, ident32)
            identbf = consts.tile([128, 128], BF16)
            make_identity(nc, identbf)

            # Preload the ACT exp table before the scalar queue's input
            # DMAs so the table load is off the first-exp critical path.
            warm = consts.tile([128, 16], F32)
            nc.scalar.activation(out=warm, in_=ident32[:, 0:16],
                                 func=EXP, scale=0.125)

            wts = big.tile([128, NP, D], BF16)
            xts = big.tile([128, NP, S], BF16)
            # Pair-0 weight columns first (they gate the first projection),
            # x^T chunks striped over three queues in k-arrival order.
            for k in range(NP):
                nc.gpsimd.dma_start(out=wts[:, k, 0:128],
                                    in_=wT[k * 128:(k + 1) * 128, 0:128])
            xeng = [nc.sync, nc.scalar, nc.gpsimd] * 2
            for k in range(NP):
                xeng[k].dma_start(out=xts[:, k, :],
                                  in_=xT[k * 128:(k + 1) * 128, :])
            bias_t = consts.tile([128, NP], F32)
            nc.gpsimd.dma_start(out=bias_t, in_=bias_d[:, :])
            mask_t = consts.tile([128, NT], F32)
            nc.gpsimd.dma_start(out=mask_t, in_=mask_d[:, :])
            for k in range(NP):
                nc.gpsimd.dma_start(out=wts[:, k, 128:D],
                                    in_=wT[k * 128:(k + 1) * 128, 128:D])

            mixbf = big.tile([128, NP, S], BF16)
            stages = [big.tile([128, H, DH], F32, name=f"stage{sj}")
                      for sj in range(NT)]

            # Persistent ping-pong Z tiles; zero halves are set once.
            zt = [[big.tile([128, S], BF16, name=f"z{q}{p}") for p in range(2)]
                  for q in range(2)]
            xlt = [[big.tile([128, NT, DH + 1], BF16, name=f"xl{q}{p}")
                    for p in range(2)] for q in range(2)]
            for q in range(2):
                olo = (1 - q) * 64
                for p in range(2):
                    nc.vector.memset(zt[q][p][olo:olo + 64, :], 0.0)
                    nc.vector.memset(xlt[q][p], 1.0)

            def prep(j):
                """Projection + Z/xl staging for head pair j."""
                pp = j % 2
                if j == 0:
                    # Pair 0 is on the critical path: run both halves
                    # concurrently (second half borrows the idle ctx slot)
                    # so the projection tracks the input DMA arrivals.
                    pms = [ps_s.tile([128, 512], F32, name="pm", bufs=1),
                           ps_s.tile([128, 512], F32, name="psc")]
                    for k in range(NP):
                        for n in range(2):
                            nc.tensor.matmul(
                                pms[n],
                                lhsT=wts[:, k, j * 128:(j + 1) * 128],
                                rhs=xts[:, k, n * 512:(n + 1) * 512],
                                start=(k == 0),
                                stop=(k == NP - 1),
                            )
                        # Warm-up transposes between the DMA-paced projection
                        # matmuls keep the HAM clock gate open through the
                        # input-streaming window.
                        for _ in range(5):
                            ptw = ps_t.tile([128, 128], BF16, name="pt")
                            nc.tensor.transpose(ptw, identbf, identbf)
                    for n in range(2):
                        nc.vector.tensor_scalar_add(
                            mixbf[:, j, n * 512:(n + 1) * 512], pms[n],
                            bias_t[:, j:j + 1]
                        )
                else:
                    for n in range(2):
                        pm = ps_s.tile([128, 512], F32, name="pm", bufs=1)
                        for k in range(NP):
                            nc.tensor.matmul(
                                pm,
                                lhsT=wts[:, k, j * 128:(j + 1) * 128],
                                rhs=xts[:, k, n * 512:(n + 1) * 512],
                                start=(k == 0),
                                stop=(k == NP - 1),
                            )
                        nc.vector.tensor_scalar_add(
                            mixbf[:, j, n * 512:(n + 1) * 512], pm,
                            bias_t[:, j:j + 1]
                        )
                zs = []
                for q in range(2):
                    z = zt[q][pp]
                    lo = q * 64
                    for n in range(2):
                        nc.vector.tensor_copy(
                            out=z[lo:lo + 64, n * 512:(n + 1) * 512],
                            in_=mixbf[lo:lo + 64, j, n * 512:(n + 1) * 512],
                        )
                    zs.append(z)
                xlns = [xlt[0][pp], xlt[1][pp]]
                for i in range(NT):
                    pt = ps_t.tile([128, 128], BF16, name="pt")
                    nc.tensor.transpose(
                        pt, mixbf[:, j, i * 128:(i + 1) * 128], identbf
                    )
                    for q in range(2):
                        nc.vector.tensor_copy(
                            out=xlns[q][:, i, 0:DH], in_=pt[:, q * 64:q * 64 + 64]
                        )
                return zs, xlns

            def scores_phase(j, q, zs, xlns=None, pcs=None):
                """Scores + exp for head (j, q); returns the U tiles. When
                pcs is given (final head), the ctx matmuls are interleaved
                so only the epilogue remains after the last exp."""
                us = []
                for i in range(NT):
                    psc = ps_s.tile([128, S], F32, name="psc")
                    for n in range(2):
                        nc.tensor.matmul(
                            psc[:, n * 512:(n + 1) * 512],
                            lhsT=zs[q][:, i * 128:(i + 1) * 128],
                            rhs=mixbf[:, j, n * 512:(n + 1) * 512],
                            start=True,
                            stop=True,
                        )
                    u = upool.tile([128, S], BF16, name="u")
                    nc.scalar.activation(
                        out=u, in_=psc, func=EXP,
                        bias=mask_t[:, i:i + 1], scale=0.125,
                    )
                    us.append(u)
                    if pcs is not None:
                        for n in range(2):
                            nc.tensor.matmul(
                                pcs[n],
                                lhsT=xlns[q][:, i, :],
                                rhs=u[:, n * 512:(n + 1) * 512],
                                start=(i == 0),
                                stop=(i == NT - 1),
                            )
                return us

            def ctx_epilogue(h, n, pc, wide=False, flush_each=False):
                """Evacuate one ctx half: transpose + normalization + stage."""
                ct = ctpool.tile([DH + 1, 512], BF16, name="ct")
                nc.vector.tensor_copy(out=ct, in_=pc)
                for sjh in range(NT // 2):
                    sj = n * 4 + sjh
                    if wide and sjh % 2 == 1:
                        po = ps_s.tile([128, DH + 1], BF16, name="psc")
                    else:
                        po = ps_t.tile([128, DH + 1], BF16, name="pt")
                    nc.tensor.transpose(
                        po,
                        ct[:, sjh * 128:(sjh + 1) * 128],
                        identbf[0:DH + 1, 0:DH + 1],
                    )
                    rcol = rpool.tile([128, 1], F32, name="rcol", bufs=24)
                    nc.vector.reciprocal(out=rcol, in_=po[:, DH:DH + 1])
                    nc.vector.tensor_scalar_mul(
                        stages[sj][:, h, :], po[:, 0:DH], rcol
                    )
                    if flush_each:
                        # Final head: stream each chunk out as soon as it
                        # is normalized so only one small DMA trails.
                        eng = [nc.sync, nc.gpsimd, nc.scalar][sj % 3]
                        eng.dma_start(
                            out=out_d[sj * 128:(sj + 1) * 128,
                                      h * 64:(h + 1) * 64],
                            in_=stages[sj][:, h:h + 1, :],
                        )

            def ctx_phase(j, q, xlns, us):
                """ctx accumulation (denominator row via the ones column) in
                two single-bank halves; transpose + normalization + stage."""
                h = 2 * j + q
                for n in range(2):
                    pc = ps_c.tile([DH + 1, 512], F32, name="pc")
                    for i in range(NT):
                        nc.tensor.matmul(
                            pc,
                            lhsT=xlns[q][:, i, :],
                            rhs=us[i][:, n * 512:(n + 1) * 512],
                            start=(i == 0),
                            stop=(i == NT - 1),
                        )
                    ctx_epilogue(h, n, pc)

            def flush(h0, h1, final=False):
                # Spread the ~600ns serial per-queue issue cost; the scalar
                # queue is only safe once the exps are done (final flush).
                engs = [nc.sync, nc.gpsimd, nc.scalar] if final else \
                    [nc.sync, nc.gpsimd]
                for sj in range(NT):
                    engs[sj % len(engs)].dma_start(
                        out=out_d[sj * 128:(sj + 1) * 128, h0 * 64:h1 * 64],
                        in_=stages[sj][:, h0:h1, :],
                    )

            state = prep(0)
            pending = None  # (j, q, xlns, us) awaiting its ctx phase
            done_heads = 0
            for j in range(NP):
                zs, xlns = state
                for q in range(2):
                    last = (j == NP - 1 and q == 1)
                    if last:
                        # Final head: keep ACT fed through the endgame. Emit
                        # the first three score/exp groups before draining the
                        # pending ctx (whose matmuls would otherwise outrank
                        # them on the PE), then accumulate this head's ctx
                        # retroactively + interleaved so only the epilogue
                        # trails the last exp.
                        us = []
                        for i in range(7):
                            psc = ps_s.tile([128, S], F32, name="psc")
                            for n in range(2):
                                nc.tensor.matmul(
                                    psc[:, n * 512:(n + 1) * 512],
                                    lhsT=zs[q][:, i * 128:(i + 1) * 128],
                                    rhs=mixbf[:, j, n * 512:(n + 1) * 512],
                                    start=True, stop=True,
                                )
                            u = upool.tile([128, S], BF16, name="u")
                            nc.scalar.activation(
                                out=u, in_=psc, func=EXP,
                                bias=mask_t[:, i:i + 1], scale=0.125,
                            )
                            us.append(u)
                        ctx_phase(*pending)
                        pending = None
                        flush(6, 10)
                        pcs = [ps_c.tile([DH + 1, 512], F32, name="pc"),
                               ps_s.tile([DH + 1, 512], F32, name="pm", bufs=1)]
                        for i in range(7):
                            for n in range(2):
                                nc.tensor.matmul(
                                    pcs[n],
                                    lhsT=xlns[q][:, i, :],
                                    rhs=us[i][:, n * 512:(n + 1) * 512],
                                    start=(i == 0), stop=False,
                                )
                        for i in range(7, NT):
                            psc = ps_s.tile([128, S], F32, name="psc")
                            for n in range(2):
                                nc.tensor.matmul(
                                    psc[:, n * 512:(n + 1) * 512],
                                    lhsT=zs[q][:, i * 128:(i + 1) * 128],
                                    rhs=mixbf[:, j, n * 512:(n + 1) * 512],
                                    start=True, stop=True,
                                )
                            u = upool.tile([128, S], BF16, name="u")
                            nc.scalar.activation(
                                out=u, in_=psc, func=EXP,
                                bias=mask_t[:, i:i + 1], scale=0.125,
                            )
                            for n in range(2):
                                nc.tensor.matmul(
                                    pcs[n],
                                    lhsT=xlns[q][:, i, :],
                                    rhs=u[:, n * 512:(n + 1) * 512],
                                    start=False, stop=(i == NT - 1),
                                )
                        flush(10, 11)
                        for n in range(2):
                            ctx_epilogue(2 * j + q, n, pcs[n], wide=True,
                                         flush_each=True)
                        continue
                    us = scores_phase(j, q, zs)
                    if q == 1:
                        # Emit the next pair's prep before the pending ctx
                        # phase: its DVE chain (evac -> z copies) then
                        # outranks the ctx epilogue work, so the next pair's
                        # scores are ready the moment this pair's exps drain.
                        state = prep(j + 1) if j + 1 < NP else None
                    if pending is not None:
                        ctx_phase(*pending)
                        done_heads += 1
                        if done_heads == 6:
                            flush(0, 6)
                    pending = (j, q, xlns, us)

    nc.compile()
    return nc


def kernel(x, attention_mask, W, b, _profile=None):
    global _CACHED_NC
    if _CACHED_NC is None:
        _CACHED_NC = build_nc()
    nc = _CACHED_NC

    x = np.asarray(x, dtype=np.float32)
    attention_mask = np.asarray(attention_mask, dtype=np.float32)
    W = np.asarray(W, dtype=np.float32)
    b = np.asarray(b, dtype=np.float32)

    import ml_dtypes

    wT = np.ascontiguousarray(W.T).astype(ml_dtypes.bfloat16)
    bias_cols = np.ascontiguousarray(b.reshape(NP, 128).T)

    in_maps = []
    for i in range(B):
        in_maps.append({
            "xT": np.ascontiguousarray(x[i].T).astype(ml_dtypes.bfloat16),
            "wT": wT,
            "bias_d": bias_cols,
            "mask_d": np.ascontiguousarray(
                attention_mask[i, 0, 0].reshape(NT, 128).T
            ),
        })

    kwargs = dict(_profile) if _profile else {}
    res = run_bass_kernel_spmd(nc, in_maps, core_ids=list(range(B)), **kwargs)
    out = np.stack([res.results[i]["out"] for i in range(B)], axis=0)
    if _profile:
        kernel.last_results = res
    return out


if __name__ == "__main__":
    rng = np.random.default_rng(0)
    x = rng.standard_normal((B, S, D), dtype=np.float32)
    m = np.zeros((B, 1, 1, S), dtype=np.float32)
    W = (rng.standard_normal((D, D), dtype=np.float32) / np.sqrt(D)).astype(np.float32)
    b = np.zeros((D,), dtype=np.float32)
    out = kernel(x, m, W, b)
    print("out", out.shape, out.dtype)



# revision 23
# speedup vs baseline: 1.6396x; 1.0070x over previous
"""BERT self-attention (B=8, S=1024, D=768, H=12) on 8 TRN2 NeuronCores.

Strategy
--------
Data-parallel over batch: core b handles batch element b (no collectives).

Per core, everything is computed in a "transposed" layout that keeps the
tensor engine's contraction dimension in the partition axis:

  1. mixedT[e, s] = sum_d W^T[d, e] * x^T[d, s] + bias[e] as bf16 matmuls
     with fp32 psum accumulation; the psum evacuation adds the
     per-partition bias and keeps bf16 for the attention stage.
  2. Q=K=V => the score matrix is symmetric: scores[t, s] = scores[s, t].
     The exp'd score tile in [t, s] layout therefore equals the transposed
     (unnormalized) probability matrix needed as the moving operand of the
     context matmul -- no probability transposes at all.
     scores tile = Z_h^T @ MIX where Z_h zero-masks the other head of the
     e-tile pair (kills cross-head terms while keeping K=128
     partition-aligned bf16 matmuls at 1 cycle/column).
     U = exp(0.125 * scores + mask[t]) is fused into the ACT psum
     evacuation (the mask enters as the per-partition bias).
  3. ctx'^T[dh, s] plus the softmax denominator as row 64 (via a constant
     ones column in the stationary operand [xl | 1] [128, 65]) accumulate
     in fp32 psum over the eight t-chunks, moving U.
  4. PE-transpose of ctx'^T 128-column chunks gives ctx[s, dh] with the
     denominator as column 64; reciprocal + per-partition scalar multiply
     normalize during the psum evacuation, writing into per-s-chunk
     staging tiles that are flushed as a few large contiguous DMAs.

Scheduling: the kernel is software-pipelined at two levels. The scalar
engine (softmax exp: 96 x [128, 1024] ACTIVATEs, ~107us) is the
steady-state bottleneck, so emission order makes everything else fill its
bubbles: scores+exp of head h are emitted before the ctx/output phase of
head h-1, and the projection/prep of e-tile pair j+1 is emitted between
them (so its evac->z chain outranks the ctx epilogues and the next pair's
scores are ready the moment this pair's exps drain). The final head's ctx
matmuls are interleaved with its own exps so only the short epilogue
trails the last ACTIVATE.

Measured on TRN2 (8 cores): ~138.2 us HW exec, rel err ~8e-3 vs the fp32
reference (bf16 operand rounding; all accumulation in fp32).
"""

import numpy as np

import concourse.bacc as bacc
import concourse.tile as tile
from concourse import mybir
from concourse.bass_utils import run_bass_kernel_spmd
from concourse.masks import make_identity

B, S, D = 8, 1024, 768
H, DH = 12, 64
NP = 6            # e-tile pairs (2 heads each)
NT = 8            # t-chunks / s-chunks of 128
F32 = mybir.dt.float32
F32R = mybir.dt.float32r
BF16 = mybir.dt.bfloat16
EXP = mybir.ActivationFunctionType.Exp

_CACHED_NC = None


def build_nc():
    nc = bacc.Bacc("TRN2", target_bir_lowering=False)

    xT = nc.dram_tensor("xT", [D, S], BF16, kind="ExternalInput")
    wT = nc.dram_tensor("wT", [D, D], BF16, kind="ExternalInput")
    bias_d = nc.dram_tensor("bias_d", [128, NP], F32, kind="ExternalInput")
    mask_d = nc.dram_tensor("mask_d", [128, NT], F32, kind="ExternalInput")
    out_d = nc.dram_tensor("out", [S, D], F32, kind="ExternalOutput")

    with tile.TileContext(nc) as tc:
        with (
            tc.tile_pool(name="consts", bufs=1) as consts,
            tc.tile_pool(name="big", bufs=1) as big,
            tc.tile_pool(name="upool", bufs=22) as upool,
            tc.tile_pool(name="ctpool", bufs=4) as ctpool,
            tc.tile_pool(name="rpool", bufs=24) as rpool,
            tc.tile_pool(name="ps_s", bufs=2, space="PSUM") as ps_s,
            tc.tile_pool(name="ps_c", bufs=1, space="PSUM") as ps_c,
            tc.tile_pool(name="ps_t", bufs=2, space="PSUM") as ps_t,
        ):
            ident32 = consts.tile([128, 128], F32)
            make_identity(nc, ident32)
            identbf = consts.tile([128, 128], BF16)
            make_identity(nc, identbf)

            # Preload the ACT exp table before the scalar queue's input
            # DMAs so the table load is off the first-exp critical path.
            warm = consts.tile([128, 16], F32)
            nc.scalar.activation(out=warm, in_=ident32[:, 0:16],
                                 func=EXP, scale=0.125)

            wts = big.tile([128, NP, D], BF16)
            xts = big.tile([128, NP, S], BF16)
            # Batched input loads (one DMA instruction each -- per-queue
            # issue cost is ~650ns, so fewer, bigger transfers): pair-0
            # weight columns first (they gate the first projection), x^T
            # striped over the three DMA-capable queues.
            nc.gpsimd.dma_start(
                out=wts[:, :, 0:128],
                in_=wT[:, 0:128].rearrange("(k p) e -> p k e", p=128))
            nc.sync.dma_start(
                out=xts[:, 0:2, :],
                in_=xT[0:256, :].rearrange("(k p) s -> p k s", p=128))
            nc.scalar.dma_start(
                out=xts[:, 2:4, :],
                in_=xT[256:512, :].rearrange("(k p) s -> p k s", p=128))
            nc.gpsimd.dma_start(
                out=xts[:, 4:6, :],
                in_=xT[512:768, :].rearrange("(k p) s -> p k s", p=128))
            bias_t = consts.tile([128, NP], F32)
            nc.gpsimd.dma_start(out=bias_t, in_=bias_d[:, :])
            mask_t = consts.tile([128, NT], F32)
            nc.gpsimd.dma_start(out=mask_t, in_=mask_d[:, :])
            nc.gpsimd.dma_start(
                out=wts[:, :, 128:D],
                in_=wT[:, 128:D].rearrange("(k p) e -> p k e", p=128))

            mixbf = big.tile([128, NP, S], BF16)
            stages = [big.tile([128, H, DH], F32, name=f"stage{sj}")
                      for sj in range(NT)]

            # Persistent ping-pong Z tiles; zero halves are set once.
            zt = [[big.tile([128, S], BF16, name=f"z{q}{p}") for p in range(2)]
                  for q in range(2)]
            xlt = [[big.tile([128, NT, DH + 1], BF16, name=f"xl{q}{p}")
                    for p in range(2)] for q in range(2)]
            for q in range(2):
                olo = (1 - q) * 64
                for p in range(2):
                    nc.vector.memset(zt[q][p][olo:olo + 64, :], 0.0)
                    nc.vector.memset(xlt[q][p], 1.0)

            def prep(j):
                """Projection + Z/xl staging for head pair j."""
                pp = j % 2
                if j == 0:
                    # Pair 0 is on the critical path: run both halves
                    # concurrently (second half borrows the idle ctx slot)
                    # so the projection tracks the input DMA arrivals.
                    pms = [ps_s.tile([128, 512], F32, name="pm", bufs=1),
                           ps_s.tile([128, 512], F32, name="psc")]
                    for k in range(NP):
                        for n in range(2):
                            nc.tensor.matmul(
                                pms[n],
                                lhsT=wts[:, k, j * 128:(j + 1) * 128],
                                rhs=xts[:, k, n * 512:(n + 1) * 512],
                                start=(k == 0),
                                stop=(k == NP - 1),
                            )
                        # Warm-up transposes between the DMA-paced projection
                        # matmuls keep the HAM clock gate open through the
                        # input-streaming window.
                        for _ in range(5):
                            ptw = ps_t.tile([128, 128], BF16, name="pt")
                            nc.tensor.transpose(ptw, identbf, identbf)
                    for n in range(2):
                        nc.vector.tensor_scalar_add(
                            mixbf[:, j, n * 512:(n + 1) * 512], pms[n],
                            bias_t[:, j:j + 1]
                        )
                else:
                    for n in range(2):
                        pm = ps_s.tile([128, 512], F32, name="pm", bufs=1)
                        for k in range(NP):
                            nc.tensor.matmul(
                                pm,
                                lhsT=wts[:, k, j * 128:(j + 1) * 128],
                                rhs=xts[:, k, n * 512:(n + 1) * 512],
                                start=(k == 0),
                                stop=(k == NP - 1),
                            )
                        nc.vector.tensor_scalar_add(
                            mixbf[:, j, n * 512:(n + 1) * 512], pm,
                            bias_t[:, j:j + 1]
                        )
                zs = []
                for q in range(2):
                    z = zt[q][pp]
                    lo = q * 64
                    for n in range(2):
                        nc.vector.tensor_copy(
                            out=z[lo:lo + 64, n * 512:(n + 1) * 512],
                            in_=mixbf[lo:lo + 64, j, n * 512:(n + 1) * 512],
                        )
                    zs.append(z)
                xlns = [xlt[0][pp], xlt[1][pp]]
                for i in range(NT):
                    pt = ps_t.tile([128, 128], BF16, name="pt")
                    nc.tensor.transpose(
                        pt, mixbf[:, j, i * 128:(i + 1) * 128], identbf
                    )
                    for q in range(2):
                        nc.vector.tensor_copy(
                            out=xlns[q][:, i, 0:DH], in_=pt[:, q * 64:q * 64 + 64]
                        )
                return zs, xlns

            def scores_phase(j, q, zs, xlns=None, pcs=None):
                """Scores + exp for head (j, q); returns the U tiles. When
                pcs is given (final head), the ctx matmuls are interleaved
                so only the epilogue remains after the last exp."""
                us = []
                for i in range(NT):
                    psc = ps_s.tile([128, S], F32, name="psc")
                    for n in range(2):
                        nc.tensor.matmul(
                            psc[:, n * 512:(n + 1) * 512],
                            lhsT=zs[q][:, i * 128:(i + 1) * 128],
                            rhs=mixbf[:, j, n * 512:(n + 1) * 512],
                            start=True,
                            stop=True,
                        )
                    u = upool.tile([128, S], BF16, name="u")
                    nc.scalar.activation(
                        out=u, in_=psc, func=EXP,
                        bias=mask_t[:, i:i + 1], scale=0.125,
                    )
                    us.append(u)
                    if pcs is not None:
                        for n in range(2):
                            nc.tensor.matmul(
                                pcs[n],
                                lhsT=xlns[q][:, i, :],
                                rhs=u[:, n * 512:(n + 1) * 512],
                                start=(i == 0),
                                stop=(i == NT - 1),
                            )
                return us

            def ctx_epilogue(h, n, pc, wide=False, flush_each=False):
                """Evacuate one ctx half: transpose + normalization + stage."""
                ct = ctpool.tile([DH + 1, 512], BF16, name="ct")
                nc.vector.tensor_copy(out=ct, in_=pc)
                for sjh in range(NT // 2):
                    sj = n * 4 + sjh
                    if wide and sjh % 2 == 1:
                        po = ps_s.tile([128, DH + 1], BF16, name="psc")
                    else:
                        po = ps_t.tile([128, DH + 1], BF16, name="pt")
                    nc.tensor.transpose(
                        po,
                        ct[:, sjh * 128:(sjh + 1) * 128],
                        identbf[0:DH + 1, 0:DH + 1],
                    )
                    rcol = rpool.tile([128, 1], F32, name="rcol", bufs=24)
                    nc.vector.reciprocal(out=rcol, in_=po[:, DH:DH + 1])
                    nc.vector.tensor_scalar_mul(
                        stages[sj][:, h, :], po[:, 0:DH], rcol
                    )
                    if flush_each:
                        # Final head: stream each chunk out as soon as it
                        # is normalized so only one small DMA trails.
                        eng = [nc.sync, nc.gpsimd, nc.scalar][sj % 3]
                        eng.dma_start(
                            out=out_d[sj * 128:(sj + 1) * 128,
                                      h * 64:(h + 1) * 64],
                            in_=stages[sj][:, h:h + 1, :],
                        )

            def ctx_phase(j, q, xlns, us):
                """ctx accumulation (denominator row via the ones column) in
                two single-bank halves; transpose + normalization + stage."""
                h = 2 * j + q
                for n in range(2):
                    pc = ps_c.tile([DH + 1, 512], F32, name="pc")
                    for i in range(NT):
                        nc.tensor.matmul(
                            pc,
                            lhsT=xlns[q][:, i, :],
                            rhs=us[i][:, n * 512:(n + 1) * 512],
                            start=(i == 0),
                            stop=(i == NT - 1),
                        )
                    ctx_epilogue(h, n, pc)

            def flush(h0, h1, final=False):
                # Spread the ~600ns serial per-queue issue cost; the scalar
                # queue is only safe once the exps are done (final flush).
                engs = [nc.sync, nc.gpsimd, nc.scalar] if final else \
                    [nc.sync, nc.gpsimd]
                for sj in range(NT):
                    engs[sj % len(engs)].dma_start(
                        out=out_d[sj * 128:(sj + 1) * 128, h0 * 64:h1 * 64],
                        in_=stages[sj][:, h0:h1, :],
                    )

            state = prep(0)
            pending = None  # (j, q, xlns, us) awaiting its ctx phase
            done_heads = 0
            for j in range(NP):
                zs, xlns = state
                for q in range(2):
                    last = (j == NP - 1 and q == 1)
                    if last:
                        # Final head: keep ACT fed through the endgame. Emit
                        # the first three score/exp groups before draining the
                        # pending ctx (whose matmuls would otherwise outrank
                        # them on the PE), then accumulate this head's ctx
                        # retroactively + interleaved so only the epilogue
                        # trails the last exp.
                        us = []
                        for i in range(7):
                            psc = ps_s.tile([128, S], F32, name="psc")
                            for n in range(2):
                                nc.tensor.matmul(
                                    psc[:, n * 512:(n + 1) * 512],
                                    lhsT=zs[q][:, i * 128:(i + 1) * 128],
                                    rhs=mixbf[:, j, n * 512:(n + 1) * 512],
                                    start=True, stop=True,
                                )
                            u = upool.tile([128, S], BF16, name="u")
                            nc.scalar.activation(
                                out=u, in_=psc, func=EXP,
                                bias=mask_t[:, i:i + 1], scale=0.125,
                            )
                            us.append(u)
                        ctx_phase(*pending)
                        pending = None
                        flush(6, 10)
                        pcs = [ps_c.tile([DH + 1, 512], F32, name="pc"),
                               ps_s.tile([DH + 1, 512], F32, name="pm", bufs=1)]
                        for i in range(7):
                            for n in range(2):
                                nc.tensor.matmul(
                                    pcs[n],
                                    lhsT=xlns[q][:, i, :],
                                    rhs=us[i][:, n * 512:(n + 1) * 512],
                                    start=(i == 0), stop=False,
                                )
                        for i in range(7, NT):
                            psc = ps_s.tile([128, S], F32, name="psc")
                            for n in range(2):
                                nc.tensor.matmul(
                                    psc[:, n * 512:(n + 1) * 512],
                                    lhsT=zs[q][:, i * 128:(i + 1) * 128],
                                    rhs=mixbf[:, j, n * 512:(n + 1) * 512],
                                    start=True, stop=True,
                                )
                            u = upool.tile([128, S], BF16, name="u")
                            nc.scalar.activation(
                                out=u, in_=psc, func=EXP,
                                bias=mask_t[:, i:i + 1], scale=0.125,
                            )
                            for n in range(2):
                                nc.tensor.matmul(
                                    pcs[n],
                                    lhsT=xlns[q][:, i, :],
                                    rhs=u[:, n * 512:(n + 1) * 512],
                                    start=False, stop=(i == NT - 1),
                                )
                        flush(10, 11)
                        for n in range(2):
                            ctx_epilogue(2 * j + q, n, pcs[n], wide=True,
                                         flush_each=True)
                        continue
                    us = scores_phase(j, q, zs)
                    if q == 1:
                        # Emit the next pair's prep before the pending ctx
                        # phase: its DVE chain (evac -> z copies) then
                        # outranks the ctx epilogue work, so the next pair's
                        # scores are ready the moment this pair's exps drain.
                        state = prep(j + 1) if j + 1 < NP else None
                    if pending is not None:
                        ctx_phase(*pending)
                        done_heads += 1
                        if done_heads == 6:
                            flush(0, 6)
                    pending = (j, q, xlns, us)

    nc.compile()
    return nc


def kernel(x, attention_mask, W, b, _profile=None):
    global _CACHED_NC
    if _CACHED_NC is None:
        _CACHED_NC = build_nc()
    nc = _CACHED_NC

    x = np.asarray(x, dtype=np.float32)
    attention_mask = np.asarray(attention_mask, dtype=np.float32)
    W = np.asarray(W, dtype=np.float32)
    b = np.asarray(b, dtype=np.float32)

    import ml_dtypes

    wT = np.ascontiguousarray(W.T).astype(ml_dtypes.bfloat16)
    bias_cols = np.ascontiguousarray(b.reshape(NP, 128).T)

    in_maps = []
    for i in range(B):
        in_maps.append({
            "xT": np.ascontiguousarray(x[i].T).astype(ml_dtypes.bfloat16),
            "wT": wT,
            "bias_d": bias_cols,
            "mask_d": np.ascontiguousarray(
                attention_mask[i, 0, 0].reshape(NT, 128).T
            ),
        })

    kwargs = dict(_profile) if _profile else {}
    res = run_bass_kernel_spmd(nc, in_maps, core_ids=list(range(B)), **kwargs)
    out = np.stack([res.results[i]["out"] for i in range(B)], axis=0)
    if _profile:
        kernel.last_results = res
    return out


if __name__ == "__main__":
    rng = np.random.default_rng(0)
    x = rng.standard_normal((B, S, D), dtype=np.float32)
    m = np.zeros((B, 1, 1, S), dtype=np.float32)
    W = (rng.standard_normal((D, D), dtype=np.float32) / np.sqrt(D)).astype(np.float32)
    b = np.zeros((D,), dtype=np.float32)
    out = kernel(x, m, W, b)
    print("out", out.shape, out.dtype)



# revision 24
# speedup vs baseline: 1.6616x; 1.0134x over previous
"""BERT self-attention (B=8, S=1024, D=768, H=12) on 8 TRN2 NeuronCores.

Strategy
--------
Data-parallel over batch: core b handles batch element b (no collectives).

Per core, everything is computed in a "transposed" layout that keeps the
tensor engine's contraction dimension in the partition axis:

  1. mixedT[e, s] = sum_d W^T[d, e] * x^T[d, s] + bias[e] as bf16 matmuls
     with fp32 psum accumulation; the psum evacuation adds the
     per-partition bias and keeps bf16 for the attention stage.
  2. Q=K=V => the score matrix is symmetric: scores[t, s] = scores[s, t].
     The exp'd score tile in [t, s] layout therefore equals the transposed
     (unnormalized) probability matrix needed as the moving operand of the
     context matmul -- no probability transposes at all.
     scores tile = Z_h^T @ MIX where Z_h zero-masks the other head of the
     e-tile pair (kills cross-head terms while keeping K=128
     partition-aligned bf16 matmuls at 1 cycle/column).
     U = exp(0.125 * scores + mask[t]) is fused into the ACT psum
     evacuation (the mask enters as the per-partition bias).
  3. ctx'^T[dh, s] plus the softmax denominator as row 64 (via a constant
     ones column in the stationary operand [xl | 1] [128, 65]) accumulate
     in fp32 psum over the eight t-chunks, moving U.
  4. PE-transpose of ctx'^T 128-column chunks gives ctx[s, dh] with the
     denominator as column 64; reciprocal + per-partition scalar multiply
     normalize during the psum evacuation, writing into per-s-chunk
     staging tiles that are flushed as a few large contiguous DMAs.

Scheduling: the kernel is software-pipelined at two levels. The scalar
engine (softmax exp: 96 x [128, 1024] ACTIVATEs, ~107us) is the
steady-state bottleneck, so emission order makes everything else fill its
bubbles: scores+exp of head h are emitted before the ctx/output phase of
head h-1, and the projection/prep of e-tile pair j+1 is emitted between
them (so its evac->z chain outranks the ctx epilogues and the next pair's
scores are ready the moment this pair's exps drain). The final head's ctx
matmuls are interleaved with its own exps so only the short epilogue
trails the last ACTIVATE.

Measured on TRN2 (8 cores): ~138.2 us HW exec, rel err ~8e-3 vs the fp32
reference (bf16 operand rounding; all accumulation in fp32).
"""

import numpy as np

import concourse.bacc as bacc
import concourse.tile as tile
from concourse import mybir
from concourse.bass_utils import run_bass_kernel_spmd
from concourse.masks import make_identity

B, S, D = 8, 1024, 768
H, DH = 12, 64
NP = 6            # e-tile pairs (2 heads each)
NT = 8            # t-chunks / s-chunks of 128
F32 = mybir.dt.float32
F32R = mybir.dt.float32r
BF16 = mybir.dt.bfloat16
EXP = mybir.ActivationFunctionType.Exp

_CACHED_NC = None


def build_nc():
    nc = bacc.Bacc("TRN2", target_bir_lowering=False)

    xT = nc.dram_tensor("xT", [D, S], BF16, kind="ExternalInput")
    wT = nc.dram_tensor("wT", [D, D], BF16, kind="ExternalInput")
    bias_d = nc.dram_tensor("bias_d", [128, NP], F32, kind="ExternalInput")
    mask_d = nc.dram_tensor("mask_d", [128, NT], F32, kind="ExternalInput")
    out_d = nc.dram_tensor("out", [S, D], F32, kind="ExternalOutput")

    with tile.TileContext(nc) as tc:
        with (
            tc.tile_pool(name="consts", bufs=1) as consts,
            tc.tile_pool(name="big", bufs=1) as big,
            tc.tile_pool(name="upool", bufs=22) as upool,
            tc.tile_pool(name="ctpool", bufs=4) as ctpool,
            tc.tile_pool(name="rpool", bufs=24) as rpool,
            tc.tile_pool(name="ps_s", bufs=2, space="PSUM") as ps_s,
            tc.tile_pool(name="ps_c", bufs=1, space="PSUM") as ps_c,
            tc.tile_pool(name="ps_t", bufs=2, space="PSUM") as ps_t,
        ):
            ident32 = consts.tile([128, 128], F32)
            make_identity(nc, ident32)
            identbf = consts.tile([128, 128], BF16)
            make_identity(nc, identbf)

            # Preload the ACT exp table before the scalar queue's input
            # DMAs so the table load is off the first-exp critical path.
            warm = consts.tile([128, 16], F32)
            nc.scalar.activation(out=warm, in_=ident32[:, 0:16],
                                 func=EXP, scale=0.125)

            wts = big.tile([128, NP, D], BF16)
            xts = big.tile([128, NP, S], BF16)
            # Batched input loads (one DMA instruction each -- per-queue
            # issue cost is ~650ns, so fewer, bigger transfers): pair-0
            # weight columns first (they gate the first projection), x^T
            # striped over the three DMA-capable queues.
            nc.gpsimd.dma_start(
                out=wts[:, :, 0:128],
                in_=wT[:, 0:128].rearrange("(k p) e -> p k e", p=128))
            nc.sync.dma_start(
                out=xts[:, 0:2, :],
                in_=xT[0:256, :].rearrange("(k p) s -> p k s", p=128))
            nc.scalar.dma_start(
                out=xts[:, 2:4, :],
                in_=xT[256:512, :].rearrange("(k p) s -> p k s", p=128))
            nc.gpsimd.dma_start(
                out=xts[:, 4:6, :],
                in_=xT[512:768, :].rearrange("(k p) s -> p k s", p=128))
            bias_t = consts.tile([128, NP], F32)
            nc.gpsimd.dma_start(out=bias_t, in_=bias_d[:, :])
            mask_t = consts.tile([128, NT], F32)
            nc.gpsimd.dma_start(out=mask_t, in_=mask_d[:, :])
            nc.gpsimd.dma_start(
                out=wts[:, :, 128:D],
                in_=wT[:, 128:D].rearrange("(k p) e -> p k e", p=128))

            mixbf = big.tile([128, NP, S], BF16)
            stages = [big.tile([128, H, DH], F32, name=f"stage{sj}")
                      for sj in range(NT)]

            # Persistent ping-pong Z tiles; zero halves are set once.
            zt = [[big.tile([128, S], BF16, name=f"z{q}{p}") for p in range(2)]
                  for q in range(2)]
            xlt = [[big.tile([128, NT, DH + 1], BF16, name=f"xl{q}{p}")
                    for p in range(2)] for q in range(2)]
            for q in range(2):
                olo = (1 - q) * 64
                for p in range(2):
                    nc.vector.memset(zt[q][p][olo:olo + 64, :], 0.0)
                    nc.vector.memset(xlt[q][p], 1.0)

            def prep(j):
                """Projection + Z/xl staging for head pair j."""
                pp = j % 2
                if j == 0:
                    # Pair 0 is on the critical path: run both halves
                    # concurrently (second half borrows the idle ctx slot)
                    # so the projection tracks the input DMA arrivals.
                    pms = [ps_s.tile([128, 512], F32, name="pm", bufs=1),
                           ps_s.tile([128, 512], F32, name="psc")]
                    for k in range(NP):
                        for n in range(2):
                            nc.tensor.matmul(
                                pms[n],
                                lhsT=wts[:, k, j * 128:(j + 1) * 128],
                                rhs=xts[:, k, n * 512:(n + 1) * 512],
                                start=(k == 0),
                                stop=(k == NP - 1),
                            )
                        # Warm-up transposes between the DMA-paced projection
                        # matmuls keep the HAM clock gate open through the
                        # input-streaming window.
                        for _ in range(5):
                            ptw = ps_t.tile([128, 128], BF16, name="pt")
                            nc.tensor.transpose(ptw, identbf, identbf)
                    for n in range(2):
                        nc.vector.tensor_scalar_add(
                            mixbf[:, j, n * 512:(n + 1) * 512], pms[n],
                            bias_t[:, j:j + 1]
                        )
                else:
                    for n in range(2):
                        pm = ps_s.tile([128, 512], F32, name="pm", bufs=1)
                        for k in range(NP):
                            nc.tensor.matmul(
                                pm,
                                lhsT=wts[:, k, j * 128:(j + 1) * 128],
                                rhs=xts[:, k, n * 512:(n + 1) * 512],
                                start=(k == 0),
                                stop=(k == NP - 1),
                            )
                        nc.vector.tensor_scalar_add(
                            mixbf[:, j, n * 512:(n + 1) * 512], pm,
                            bias_t[:, j:j + 1]
                        )
                zs = []
                for q in range(2):
                    z = zt[q][pp]
                    lo = q * 64
                    for n in range(2):
                        nc.vector.tensor_copy(
                            out=z[lo:lo + 64, n * 512:(n + 1) * 512],
                            in_=mixbf[lo:lo + 64, j, n * 512:(n + 1) * 512],
                        )
                    zs.append(z)
                xlns = [xlt[0][pp], xlt[1][pp]]
                for i in range(NT):
                    pt = ps_t.tile([128, 128], BF16, name="pt")
                    nc.tensor.transpose(
                        pt, mixbf[:, j, i * 128:(i + 1) * 128], identbf
                    )
                    for q in range(2):
                        nc.vector.tensor_copy(
                            out=xlns[q][:, i, 0:DH], in_=pt[:, q * 64:q * 64 + 64]
                        )
                return zs, xlns

            def scores_phase(j, q, zs, xlns=None, pcs=None):
                """Scores + exp for head (j, q); returns the U tiles. When
                pcs is given (final head), the ctx matmuls are interleaved
                so only the epilogue remains after the last exp."""
                us = []
                for i in range(NT):
                    psc = ps_s.tile([128, S], F32, name="psc")
                    for n in range(2):
                        nc.tensor.matmul(
                            psc[:, n * 512:(n + 1) * 512],
                            lhsT=zs[q][:, i * 128:(i + 1) * 128],
                            rhs=mixbf[:, j, n * 512:(n + 1) * 512],
                            start=True,
                            stop=True,
                        )
                    u = upool.tile([128, S], BF16, name="u")
                    nc.scalar.activation(
                        out=u, in_=psc, func=EXP,
                        bias=mask_t[:, i:i + 1], scale=0.125,
                    )
                    us.append(u)
                    if pcs is not None:
                        for n in range(2):
                            nc.tensor.matmul(
                                pcs[n],
                                lhsT=xlns[q][:, i, :],
                                rhs=u[:, n * 512:(n + 1) * 512],
                                start=(i == 0),
                                stop=(i == NT - 1),
                            )
                return us

            def ctx_epilogue(h, n, pc, wide=False, flush_each=False):
                """Evacuate one ctx half: transpose + normalization + stage."""
                ct = ctpool.tile([DH + 1, 512], BF16, name="ct")
                nc.vector.tensor_copy(out=ct, in_=pc)
                for sjh in range(NT // 2):
                    sj = n * 4 + sjh
                    if wide and sjh % 2 == 1:
                        po = ps_s.tile([128, DH + 1], BF16, name="psc")
                    else:
                        po = ps_t.tile([128, DH + 1], BF16, name="pt")
                    nc.tensor.transpose(
                        po,
                        ct[:, sjh * 128:(sjh + 1) * 128],
                        identbf[0:DH + 1, 0:DH + 1],
                    )
                    rcol = rpool.tile([128, 1], F32, name="rcol", bufs=24)
                    nc.vector.reciprocal(out=rcol, in_=po[:, DH:DH + 1])
                    nc.vector.tensor_scalar_mul(
                        stages[sj][:, h, :], po[:, 0:DH], rcol
                    )
                    if flush_each:
                        # Final head: stream each chunk out as soon as it
                        # is normalized so only one small DMA trails.
                        eng = [nc.sync, nc.gpsimd, nc.scalar][sj % 3]
                        eng.dma_start(
                            out=out_d[sj * 128:(sj + 1) * 128,
                                      h * 64:(h + 1) * 64],
                            in_=stages[sj][:, h:h + 1, :],
                        )

            def ctx_phase(j, q, xlns, us):
                """ctx accumulation (denominator row via the ones column) in
                two single-bank halves; transpose + normalization + stage."""
                h = 2 * j + q
                for n in range(2):
                    pc = ps_c.tile([DH + 1, 512], F32, name="pc")
                    for i in range(NT):
                        nc.tensor.matmul(
                            pc,
                            lhsT=xlns[q][:, i, :],
                            rhs=us[i][:, n * 512:(n + 1) * 512],
                            start=(i == 0),
                            stop=(i == NT - 1),
                        )
                    ctx_epilogue(h, n, pc)

            def flush(h0, h1, final=False):
                # Spread the ~600ns serial per-queue issue cost; the scalar
                # queue is only safe once the exps are done (final flush).
                engs = [nc.sync, nc.gpsimd, nc.scalar] if final else \
                    [nc.sync, nc.gpsimd]
                for sj in range(NT):
                    engs[sj % len(engs)].dma_start(
                        out=out_d[sj * 128:(sj + 1) * 128, h0 * 64:h1 * 64],
                        in_=stages[sj][:, h0:h1, :],
                    )

            state = prep(0)
            pending = None  # (j, q, xlns, us) awaiting its ctx phase
            done_heads = 0
            for j in range(NP):
                zs, xlns = state
                for q in range(2):
                    last = (j == NP - 1 and q == 1)
                    if last:
                        # Final head: keep ACT fed through the endgame. Emit
                        # the first three score/exp groups before draining the
                        # pending ctx (whose matmuls would otherwise outrank
                        # them on the PE), then accumulate this head's ctx
                        # retroactively + interleaved so only the epilogue
                        # trails the last exp.
                        us = []
                        for i in range(NT):
                            psc = ps_s.tile([128, S], F32, name="psc")
                            for n in range(2):
                                nc.tensor.matmul(
                                    psc[:, n * 512:(n + 1) * 512],
                                    lhsT=zs[q][:, i * 128:(i + 1) * 128],
                                    rhs=mixbf[:, j, n * 512:(n + 1) * 512],
                                    start=True, stop=True,
                                )
                            u = upool.tile([128, S], BF16, name="u")
                            nc.scalar.activation(
                                out=u, in_=psc, func=EXP,
                                bias=mask_t[:, i:i + 1], scale=0.125,
                            )
                            us.append(u)
                        ctx_phase(*pending)
                        pending = None
                        flush(6, 10)
                        pcs = [ps_c.tile([DH + 1, 512], F32, name="pc"),
                               ps_s.tile([DH + 1, 512], F32, name="pm", bufs=1)]
                        for i in range(NT):
                            for n in range(2):
                                nc.tensor.matmul(
                                    pcs[n],
                                    lhsT=xlns[q][:, i, :],
                                    rhs=us[i][:, n * 512:(n + 1) * 512],
                                    start=(i == 0), stop=(i == NT - 1),
                                )
                        flush(10, 11)
                        for n in range(2):
                            ctx_epilogue(2 * j + q, n, pcs[n], wide=True,
                                         flush_each=True)
                        continue
                    us = scores_phase(j, q, zs)
                    if q == 1:
                        # Emit the next pair's prep before the pending ctx
                        # phase: its DVE chain (evac -> z copies) then
                        # outranks the ctx epilogue work, so the next pair's
                        # scores are ready the moment this pair's exps drain.
                        state = prep(j + 1) if j + 1 < NP else None
                    if pending is not None:
                        ctx_phase(*pending)
                        done_heads += 1
                        if done_heads == 6:
                            flush(0, 6)
                    pending = (j, q, xlns, us)

    nc.compile()
    return nc


def kernel(x, attention_mask, W, b, _profile=None):
    global _CACHED_NC
    if _CACHED_NC is None:
        _CACHED_NC = build_nc()
    nc = _CACHED_NC

    x = np.asarray(x, dtype=np.float32)
    attention_mask = np.asarray(attention_mask, dtype=np.float32)
    W = np.asarray(W, dtype=np.float32)
    b = np.asarray(b, dtype=np.float32)

    import ml_dtypes

    wT = np.ascontiguousarray(W.T).astype(ml_dtypes.bfloat16)
    bias_cols = np.ascontiguousarray(b.reshape(NP, 128).T)

    in_maps = []
    for i in range(B):
        in_maps.append({
            "xT": np.ascontiguousarray(x[i].T).astype(ml_dtypes.bfloat16),
            "wT": wT,
            "bias_d": bias_cols,
            "mask_d": np.ascontiguousarray(
                attention_mask[i, 0, 0].reshape(NT, 128).T
            ),
        })

    kwargs = dict(_profile) if _profile else {}
    res = run_bass_kernel_spmd(nc, in_maps, core_ids=list(range(B)), **kwargs)
    out = np.stack([res.results[i]["out"] for i in range(B)], axis=0)
    if _profile:
        kernel.last_results = res
    return out


if __name__ == "__main__":
    rng = np.random.default_rng(0)
    x = rng.standard_normal((B, S, D), dtype=np.float32)
    m = np.zeros((B, 1, 1, S), dtype=np.float32)
    W = (rng.standard_normal((D, D), dtype=np.float32) / np.sqrt(D)).astype(np.float32)
    b = np.zeros((D,), dtype=np.float32)
    out = kernel(x, m, W, b)
    print("out", out.shape, out.dtype)

